# revision 5
# baseline (speedup 1.0000x reference)
"""CoherenceNet additive-attention kernel for one TRN2 chip (8 NeuronCores).

Problem (per reference):
  score_s[n,m] = ws_s . tanh(A_s[n,:] + B_s[m,:]) + bs_s    (A = stmts@Wc1.T + bc, B = attender@Wc2.T)
  w_ss = softmax over n;  ctx_s = w_ss.T @ stmts             (same for eres)
  att = tanh([attender, ctx_s, ctx_e] @ W_lin.T + b_lin);  out = att @ W_coh.T + b_coh

Sharding: attender (M=1024) axis split across 8 cores (128 attenders per core);
all attendee tensors + weights replicated. No collectives needed - softmax
reduction is over attendees, local to each attender column.

Per-core layout strategy (h/k on partitions for the big middle phase):
  A_sT [h=128, n]   = Wc1 @ stmts.T          (PE, via transposed operands)
  B_sT [h=128, m]   = Wc2 @ att.T + bc       (bias folded here; bs_* cancels in softmax)
  per m: X = A_sT + B_sT[:,m]                (DVE tensor_scalar add, 2x fp32->bf16)
         T = tanh(X)                         (ACT, bf16 - the ~167us/core ACT floor)
         score rows via one-hot ws matmul    (PE bf16, accumulating into PSUM [m,n] layout)
  softmax over n batched across m directly on [m=128, n] PSUM scores (exp without
  max subtraction - scores are bounded by ||ws||_1 so fp32 exp cannot overflow);
  ctx via PE-transposed weights; fp32 MLP head.
"""

import numpy as np

H = 128
NS = 1024
NE = 512
M = 1024
N_CORES = 8
M_LOC = M // N_CORES  # 128 attenders per core
G = 8  # tanh slab batching (m's per ACTIVATE)

_CACHE = {}


def _build_nc():
    import concourse.bacc as bacc
    import concourse.mybir as mybir
    import concourse.tile as tile
    from concourse import masks

    f32 = mybir.dt.float32
    bf16 = mybir.dt.bfloat16
    AF = mybir.ActivationFunctionType
    AX = mybir.AxisListType

    nc = bacc.Bacc(
        "TRN2",
        target_bir_lowering=False,
        debug=False,
        enable_asserts=False,
        num_devices=N_CORES,
    )

    din = {}
    for name, shape in [
        ("attendee_stmts", [NS, H]),
        ("attendee_eres", [NE, H]),
        ("attender", [M_LOC, H]),
        ("Wc_s", [H, 2 * H]),
        ("bc_s", [H]),
        ("ws_s", [H]),
        ("bs_s", [1]),
        ("Wc_e", [H, 2 * H]),
        ("bc_e", [H]),
        ("ws_e", [H]),
        ("bs_e", [1]),
        ("W_lin", [H, 3 * H]),
        ("b_lin", [H]),
        ("W_coh", [1, H]),
        ("b_coh", [1]),
    ]:
        din[name] = nc.dram_tensor(name, shape, f32, kind="ExternalInput").ap()
    out_d = nc.dram_tensor("out", [M_LOC, 1], f32, kind="ExternalOutput").ap()

    NCH_S = NS // 128  # 8 stmt chunks
    NCH_E = NE // 128  # 4 ere chunks
    NTOT = NS + NE  # 1536

    with tile.TileContext(nc) as tc:
        with (
            tc.tile_pool(name="const", bufs=1) as const,
            tc.tile_pool(name="xbuf", bufs=2) as xpool,
            tc.tile_pool(name="tbuf", bufs=2) as tpool,
            tc.tile_pool(name="work", bufs=1) as work,
            tc.tile_pool(name="ps_score", bufs=1, space="PSUM") as ps_score,
            tc.tile_pool(name="ps_tmp", bufs=2, space="PSUM") as ps_tmp,
            tc.tile_pool(name="ps_acc", bufs=1, space="PSUM") as ps_acc,
        ):
            # identity for PE transposes - first, nothing depends on DMA
            ident = const.tile([128, 128], f32)
            masks.make_identity(nc, ident[:])

            def transpose_to(dst_ap, src_ap, copy_eng):
                pt = ps_tmp.tile([128, 128], f32, tag="tmp")
                nc.tensor.transpose(pt[:], src_ap, ident[:])
                if copy_eng == "act":
                    nc.scalar.copy(dst_ap, pt[:])
                else:
                    nc.vector.tensor_copy(dst_ap, pt[:])

            # ---------- critical-path loads + transposes (stmt A path) ------
            stmts = const.tile([128, NCH_S, H], f32)
            stmtsT = const.tile([128, NCH_S, 128], f32)  # [k, n]
            stmts_r = din["attendee_stmts"].rearrange("(c p) h -> p c h", p=128)
            for c in range(NCH_S):
                nc.sync.dma_start(stmts[:, c, :], stmts_r[:, c, :])
                transpose_to(stmtsT[:, c, :], stmts[:, c, :], "act")

            wc_s = const.tile([128, 2 * H], f32)
            nc.sync.dma_start(wc_s[:], din["Wc_s"])
            wc1T_s = const.tile([128, 128], f32)  # [k, h]
            transpose_to(wc1T_s[:], wc_s[:, 0:H], "act")
            wc2T_s = const.tile([128, 128], f32)
            transpose_to(wc2T_s[:], wc_s[:, H : 2 * H], "act")

            att = const.tile([128, H], f32)
            nc.sync.dma_start(att[:], din["attender"])
            attT = const.tile([128, 128], f32)  # [k, m]
            transpose_to(attT[:], att[:], "act")

            def load_col(name):
                t = const.tile([128, 1], f32, tag=f"col_{name}")
                nc.sync.dma_start(
                    t[:], din[name].rearrange("(p one) -> p one", one=1)
                )
                return t

            bc_s_c = load_col("bc_s")
            bc_e_c = load_col("bc_e")
            ws_s_c = load_col("ws_s")
            ws_e_c = load_col("ws_e")

            # A_sT[h, n] = sum_k Wc1T[k,h] * stmtsT[k,n]
            a_sT = const.tile([128, NS], f32)
            stmtsT_flat = stmtsT[:].rearrange("p c h -> p (c h)")
            for j in range(NS // 512):
                pa = ps_tmp.tile([128, 512], f32, tag="tmp")
                nc.tensor.matmul(
                    pa[:], wc1T_s[:], stmtsT_flat[:, j * 512 : (j + 1) * 512],
                    start=True, stop=True,
                )
                nc.scalar.copy(a_sT[:, j * 512 : (j + 1) * 512], pa[:])
            # B'_sT[h, m] = Wc2T.T @ attT + bc_s
            b_sT = const.tile([128, M_LOC], f32)
            pb = ps_tmp.tile([128, 128], f32, tag="tmp")
            nc.tensor.matmul(pb[:], wc2T_s[:], attT[:], start=True, stop=True)
            nc.vector.tensor_scalar_add(b_sT[:], pb[:], bc_s_c[:])

            # ---------- ere A path ----------
            eres = const.tile([128, NCH_E, H], f32)
            eresT = const.tile([128, NCH_E, 128], f32)
            eres_r = din["attendee_eres"].rearrange("(c p) h -> p c h", p=128)
            for c in range(NCH_E):
                nc.sync.dma_start(eres[:, c, :], eres_r[:, c, :])
                transpose_to(eresT[:, c, :], eres[:, c, :], "act")
            wc_e = const.tile([128, 2 * H], f32)
            nc.sync.dma_start(wc_e[:], din["Wc_e"])
            wc1T_e = const.tile([128, 128], f32)
            transpose_to(wc1T_e[:], wc_e[:, 0:H], "act")
            wc2T_e = const.tile([128, 128], f32)
            transpose_to(wc2T_e[:], wc_e[:, H : 2 * H], "act")

            a_eT = const.tile([128, NE], f32)
            pa = ps_tmp.tile([128, 512], f32, tag="tmp")
            nc.tensor.matmul(
                pa[:], wc1T_e[:], eresT[:].rearrange("p c h -> p (c h)"),
                start=True, stop=True,
            )
            nc.scalar.copy(a_eT[:], pa[:])
            b_eT = const.tile([128, M_LOC], f32)
            pb = ps_tmp.tile([128, 128], f32, tag="tmp")
            nc.tensor.matmul(pb[:], wc2T_e[:], attT[:], start=True, stop=True)
            nc.vector.tensor_scalar_add(b_eT[:], pb[:], bc_e_c[:])

            # one-hot ws weight blocks (bf16): oh[:, j, :] has ws in column j
            oh_s = const.tile([128, 32, 32], bf16)
            oh_e = const.tile([128, 32, 32], bf16)
            nc.vector.memset(oh_s[:], 0.0)
            nc.vector.memset(oh_e[:], 0.0)
            for j in range(32):
                nc.vector.tensor_copy(oh_s[:, j, j : j + 1], ws_s_c[:])
                nc.vector.tensor_copy(oh_e[:, j, j : j + 1], ws_e_c[:])

            # ---------------- main loop: tanh slabs + score matmuls ---------
            # scores end up [m=128, n] in PSUM: cols 0:1024 stmt, 1024:1536 ere
            score = ps_score.tile([128, NTOT], f32)
            for grp in range(M_LOC // G):
                xb = xpool.tile([128, G, NTOT], bf16)
                tb = tpool.tile([128, G, NTOT], bf16)
                for g in range(G):
                    m = grp * G + g
                    nc.vector.tensor_scalar_add(
                        xb[:, g, 0:NS], a_sT[:], b_sT[:, m : m + 1]
                    )
                    nc.vector.tensor_scalar_add(
                        xb[:, g, NS:NTOT], a_eT[:], b_eT[:, m : m + 1]
                    )
                nc.scalar.activation(tb[:], xb[:], AF.Tanh)
                for g in range(G):
                    m = grp * G + g
                    jb, col = divmod(m, 32)
                    st = col == 0
                    sp = col == 31
                    rows = slice(32 * jb, 32 * jb + 32)
                    tp = (0, 32 * jb)
                    nc.tensor.matmul(
                        score[rows, 0:512], oh_s[:, col, :], tb[:, g, 0:512],
                        start=st, stop=sp, tile_position=tp,
                    )
                    nc.tensor.matmul(
                        score[rows, 512:1024], oh_s[:, col, :], tb[:, g, 512:1024],
                        start=st, stop=sp, tile_position=tp,
                    )
                    nc.tensor.matmul(
                        score[rows, 1024:1536], oh_e[:, col, :], tb[:, g, 1024:1536],
                        start=st, stop=sp, tile_position=tp,
                    )

            # ---------- tail-only loads (issued late on purpose) ----------
            wlin = const.tile([128, 3 * H], f32)
            nc.sync.dma_start(wlin[:], din["W_lin"])
            wlinT = const.tile([128, 3, 128], f32)  # [k, a] chunks
            for c in range(3):
                transpose_to(wlinT[:, c, :], wlin[:, c * 128 : (c + 1) * 128], "act")
            blin_c = load_col("b_lin")
            wcoh_c = const.tile([128, 1], f32)
            nc.sync.dma_start(wcoh_c[:], din["W_coh"].rearrange("one p -> p one"))
            bcoh_c = const.tile([1, 1], f32)
            nc.sync.dma_start(bcoh_c[:], din["b_coh"].rearrange("(o t) -> o t", o=1))

            # ---------------- softmax over n (batched across all m) ---------
            # no max subtraction: |score| <= ||ws||_1 ~ 9, exp() safe in fp32
            e_all = work.tile([128, NTOT], f32)
            nc.scalar.activation(e_all[:], score[:], AF.Exp)
            sum_s = work.tile([128, 1], f32)
            nc.vector.reduce_sum(sum_s[:], e_all[:, 0:NS], axis=AX.X)
            rs_s = work.tile([128, 1], f32)
            nc.vector.reciprocal(rs_s[:], sum_s[:])
            sum_e = work.tile([128, 1], f32)
            nc.vector.reduce_sum(sum_e[:], e_all[:, NS:NTOT], axis=AX.X)
            rs_e = work.tile([128, 1], f32)
            nc.vector.reciprocal(rs_e[:], sum_e[:])

            # transpose unnormalized weights to [n, m] for ctx matmuls
            esT = work.tile([128, NCH_S, 128], f32)
            for c in range(NCH_S):
                transpose_to(
                    esT[:, c, :], e_all[:, c * 128 : (c + 1) * 128],
                    "act" if c % 2 else "dve",
                )
            eeT = work.tile([128, NCH_E, 128], f32)
            for c in range(NCH_E):
                transpose_to(
                    eeT[:, c, :], e_all[:, NS + c * 128 : NS + (c + 1) * 128],
                    "act" if c % 2 else "dve",
                )

            # ctx[m, h] = (1/sum[m]) * sum_n e[n,m] * attendee[n,h]
            ctx_s_ps = ps_acc.tile([128, 128], f32, tag="ctx_s")
            for c in range(NCH_S):
                nc.tensor.matmul(
                    ctx_s_ps[:], esT[:, c, :], stmts[:, c, :],
                    start=(c == 0), stop=(c == NCH_S - 1),
                )
            ctx_s = work.tile([128, 128], f32)
            nc.vector.tensor_scalar_mul(ctx_s[:], ctx_s_ps[:], rs_s[:])
            ctx_e_ps = ps_acc.tile([128, 128], f32, tag="ctx_e")
            for c in range(NCH_E):
                nc.tensor.matmul(
                    ctx_e_ps[:], eeT[:, c, :], eres[:, c, :],
                    start=(c == 0), stop=(c == NCH_E - 1),
                )
            ctx_e = work.tile([128, 128], f32)
            nc.vector.tensor_scalar_mul(ctx_e[:], ctx_e_ps[:], rs_e[:])

            ctxsT = work.tile([128, 128], f32)
            transpose_to(ctxsT[:], ctx_s[:], "act")
            ctxeT = work.tile([128, 128], f32)
            transpose_to(ctxeT[:], ctx_e[:], "dve")

            # att_vec[a, m] = tanh(sum_k W_linT[k,a] * feats_T[k,m] + b_lin[a])
            av_ps = ps_acc.tile([128, 128], f32, tag="av")
            nc.tensor.matmul(av_ps[:], wlinT[:, 0, :], attT[:], start=True, stop=False)
            nc.tensor.matmul(av_ps[:], wlinT[:, 1, :], ctxsT[:], start=False, stop=False)
            nc.tensor.matmul(av_ps[:], wlinT[:, 2, :], ctxeT[:], start=False, stop=True)
            av = work.tile([128, 128], f32)
            nc.scalar.activation(av[:], av_ps[:], AF.Tanh, bias=blin_c[:])

            # coherence[m] = sum_a W_coh[a] * av[a, m] + b_coh
            coh_ps = ps_acc.tile([1, 128], f32, tag="ctx_s")
            nc.tensor.matmul(coh_ps[:], wcoh_c[:], av[:], start=True, stop=True)
            coh = work.tile([1, 128], f32)
            nc.vector.tensor_scalar_add(coh[:], coh_ps[:], bcoh_c[:])

            nc.sync.dma_start(out_d.rearrange("m one -> one m"), coh[:])

    nc.compile()
    return nc


def _get_nc():
    if "nc" not in _CACHE:
        _CACHE["nc"] = _build_nc()
    return _CACHE["nc"]


def kernel(**inputs):
    from concourse.bass_utils import run_bass_kernel_spmd

    nc = _get_nc()
    full = {k: np.ascontiguousarray(np.asarray(v, dtype=np.float32)) for k, v in inputs.items()}
    in_maps = []
    for i in range(N_CORES):
        m = dict(full)
        m["attender"] = np.ascontiguousarray(
            full["attender"][i * M_LOC : (i + 1) * M_LOC]
        )
        in_maps.append(m)
    res = run_bass_kernel_spmd(nc, in_maps, core_ids=list(range(N_CORES)))
    out = np.concatenate([res.results[i]["out"] for i in range(N_CORES)], axis=0)
    return out.astype(np.float32)


# revision 10
# speedup vs baseline: 1.2607x; 1.2607x over previous
"""CoherenceNet additive-attention kernel for one TRN2 chip (8 NeuronCores).

Problem (per reference):
  score_s[n,m] = ws_s . tanh(A_s[n,:] + B_s[m,:]) + bs_s    (A = stmts@Wc1.T + bc, B = attender@Wc2.T)
  w_ss = softmax over n;  ctx_s = w_ss.T @ stmts             (same for eres)
  att = tanh([attender, ctx_s, ctx_e] @ W_lin.T + b_lin);  out = att @ W_coh.T + b_coh

Sharding: attender (M=1024) axis split across 8 cores (128 attenders per core);
all attendee tensors + weights replicated. No collectives needed - softmax
reduction is over attendees, local to each attender column.

Per-core layout strategy (h/k on partitions for the big middle phase):
  A_sT [h=128, n]   = Wc1 @ stmts.T          (PE, via transposed operands)
  B_sT [h=128, m]   = Wc2 @ att.T + bc       (bias folded here; bs_* cancels in softmax)
  per m: X = A_sT + B_sT[:,m]                (DVE tensor_scalar add, 2x fp32->bf16)
         T = tanh(X)                         (ACT, bf16 - the ~167us/core ACT floor)
         score rows via one-hot ws matmul    (PE bf16, accumulating into PSUM [m,n] layout)
  softmax over n batched across m directly on [m=128, n] PSUM scores (exp without
  max subtraction - scores are bounded by ||ws||_1 so fp32 exp cannot overflow);
  ctx via PE-transposed weights; fp32 MLP head.
"""

import numpy as np

H = 128
NS = 1024
NE = 512
M = 1024
N_CORES = 8
M_LOC = M // N_CORES  # 128 attenders per core
G = 4  # tanh slab batching (m's per ACTIVATE)

_CACHE = {}


def _build_nc():
    import concourse.bacc as bacc
    import concourse.mybir as mybir
    import concourse.tile as tile
    from concourse import masks

    f32 = mybir.dt.float32
    bf16 = mybir.dt.bfloat16
    AF = mybir.ActivationFunctionType
    AX = mybir.AxisListType

    nc = bacc.Bacc(
        "TRN2",
        target_bir_lowering=False,
        debug=False,
        enable_asserts=False,
        num_devices=N_CORES,
    )

    din = {}
    for name, shape in [
        ("attendee_stmts", [NS, H]),
        ("attendee_eres", [NE, H]),
        ("attender", [M_LOC, H]),
        ("Wc_s", [H, 2 * H]),
        ("bc_s", [H]),
        ("ws_s", [H]),
        ("bs_s", [1]),
        ("Wc_e", [H, 2 * H]),
        ("bc_e", [H]),
        ("ws_e", [H]),
        ("bs_e", [1]),
        ("W_lin", [H, 3 * H]),
        ("b_lin", [H]),
        ("W_coh", [1, H]),
        ("b_coh", [1]),
    ]:
        din[name] = nc.dram_tensor(name, shape, f32, kind="ExternalInput").ap()
    out_d = nc.dram_tensor("out", [M_LOC, 1], f32, kind="ExternalOutput").ap()

    NCH_S = NS // 128  # 8 stmt chunks
    NCH_E = NE // 128  # 4 ere chunks
    NTOT = NS + NE  # 1536

    with tile.TileContext(nc) as tc:
        with (
            tc.tile_pool(name="const", bufs=1) as const,
            tc.tile_pool(name="xbuf", bufs=3) as xpool,
            tc.tile_pool(name="tbuf", bufs=3) as tpool,
            tc.tile_pool(name="work", bufs=1) as work,
            tc.tile_pool(name="ps_score", bufs=1, space="PSUM") as ps_score,
            tc.tile_pool(name="ps_tmp", bufs=2, space="PSUM") as ps_tmp,
            tc.tile_pool(name="ps_acc", bufs=1, space="PSUM") as ps_acc,
        ):
            # identity for PE transposes - first, nothing depends on DMA
            ident = const.tile([128, 128], f32)
            masks.make_identity(nc, ident[:])

            def transpose_to(dst_ap, src_ap, copy_eng):
                pt = ps_tmp.tile([128, 128], f32, tag="tmp")
                nc.tensor.transpose(pt[:], src_ap, ident[:])
                if copy_eng == "act":
                    nc.scalar.copy(dst_ap, pt[:])
                else:
                    nc.vector.tensor_copy(dst_ap, pt[:])

            # ---------- critical-path loads (big DMAs on SP queue; each
            # dma_start costs ~650ns of serialized SP issue time, so few+big) --
            stmts = const.tile([128, NCH_S, H], f32)
            stmtsT = const.tile([128, NCH_S, 128], f32)  # [k, n]
            stmts_r = din["attendee_stmts"].rearrange("(c p) h -> p c h", p=128)
            nc.sync.dma_start(stmts[:], stmts_r)
            wc_s = const.tile([128, 2 * H], f32)
            nc.sync.dma_start(wc_s[:], din["Wc_s"])
            att = const.tile([128, H], f32)
            nc.sync.dma_start(att[:], din["attender"])
            eres = const.tile([128, NCH_E, H], f32)
            eres_r = din["attendee_eres"].rearrange("(c p) h -> p c h", p=128)
            nc.sync.dma_start(eres[:], eres_r)
            wc_e = const.tile([128, 2 * H], f32)
            nc.sync.dma_start(wc_e[:], din["Wc_e"])

            def load_col(name):
                t = const.tile([128, 1], f32, tag=f"col_{name}")
                nc.sync.dma_start(
                    t[:], din[name].rearrange("(p one) -> p one", one=1)
                )
                return t

            bc_s_c = load_col("bc_s")
            bc_e_c = load_col("bc_e")
            ws_s_c = load_col("ws_s")
            ws_e_c = load_col("ws_e")

            # ---------- transposes (stmt A path first) ----------
            for c in range(NCH_S):
                transpose_to(stmtsT[:, c, :], stmts[:, c, :], "act" if c % 2 else "dve")
            wc1T_s = const.tile([128, 128], f32)  # [k, h]
            transpose_to(wc1T_s[:], wc_s[:, 0:H], "dve")
            wc2T_s = const.tile([128, 128], f32)
            transpose_to(wc2T_s[:], wc_s[:, H : 2 * H], "dve")
            attT = const.tile([128, 128], f32)  # [k, m]
            transpose_to(attT[:], att[:], "dve")

            # A_sT[h, n] = sum_k Wc1T[k,h] * stmtsT[k,n]
            a_sT = const.tile([128, NS], f32)
            stmtsT_flat = stmtsT[:].rearrange("p c h -> p (c h)")
            for j in range(NS // 512):
                pa = ps_tmp.tile([128, 512], f32, tag="tmp")
                nc.tensor.matmul(
                    pa[:], wc1T_s[:], stmtsT_flat[:, j * 512 : (j + 1) * 512],
                    start=True, stop=True,
                )
                nc.scalar.copy(a_sT[:, j * 512 : (j + 1) * 512], pa[:])
            # B'_sT[h, m] = Wc2T.T @ attT + bc_s
            b_sT = const.tile([128, M_LOC], f32)
            pb = ps_tmp.tile([128, 128], f32, tag="tmp")
            nc.tensor.matmul(pb[:], wc2T_s[:], attT[:], start=True, stop=True)
            nc.vector.tensor_scalar_add(b_sT[:], pb[:], bc_s_c[:])

            # ---------- ere A path ----------
            eresT = const.tile([128, NCH_E, 128], f32)
            for c in range(NCH_E):
                transpose_to(eresT[:, c, :], eres[:, c, :], "act" if c % 2 else "dve")
            wc1T_e = const.tile([128, 128], f32)
            transpose_to(wc1T_e[:], wc_e[:, 0:H], "dve")
            wc2T_e = const.tile([128, 128], f32)
            transpose_to(wc2T_e[:], wc_e[:, H : 2 * H], "dve")

            a_eT = const.tile([128, NE], f32)
            pa = ps_tmp.tile([128, 512], f32, tag="tmp")
            nc.tensor.matmul(
                pa[:], wc1T_e[:], eresT[:].rearrange("p c h -> p (c h)"),
                start=True, stop=True,
            )
            nc.scalar.copy(a_eT[:], pa[:])
            b_eT = const.tile([128, M_LOC], f32)
            pb = ps_tmp.tile([128, 128], f32, tag="tmp")
            nc.tensor.matmul(pb[:], wc2T_e[:], attT[:], start=True, stop=True)
            nc.vector.tensor_scalar_add(b_eT[:], pb[:], bc_e_c[:])

            # one-hot ws strips (bf16): z[:, 0:31]=0, z[:, 31]=ws, z[:, 32:63]=0.
            # The one-hot [128, 32] weight matrix with ws in column c is the
            # contiguous slice z[:, 31-c : 63-c] - no per-column build needed.
            z_s = const.tile([128, 63], bf16)
            z_e = const.tile([128, 63], bf16)
            nc.vector.memset(z_s[:], 0.0)
            nc.vector.memset(z_e[:], 0.0)
            nc.vector.tensor_copy(z_s[:, 31:32], ws_s_c[:])
            nc.vector.tensor_copy(z_e[:, 31:32], ws_e_c[:])

            # ---------------- main loop: tanh slabs + score matmuls ---------
            # scores end up [m=128, n] in PSUM: cols 0:1024 stmt, 1024:1536 ere
            # ramped group sizes: small groups at start (first tanh issues after
            # only 2 adds) and at end (last score MMs trail a small tanh)
            GROUPS = [1, 1, 2] + [G] * ((M_LOC - 8) // G) + [2, 1, 1]
            assert sum(GROUPS) == M_LOC
            score = ps_score.tile([128, NTOT], f32)
            m0 = 0
            for gi, gsz in enumerate(GROUPS):
                xb = xpool.tile([128, gsz, NTOT], bf16, tag="xb")
                tb = tpool.tile([128, gsz, NTOT], bf16, tag="tb")
                for g in range(gsz):
                    m = m0 + g
                    nc.vector.tensor_scalar_add(
                        xb[:, g, 0:NS], a_sT[:], b_sT[:, m : m + 1]
                    )
                    nc.vector.tensor_scalar_add(
                        xb[:, g, NS:NTOT], a_eT[:], b_eT[:, m : m + 1]
                    )
                nc.scalar.activation(tb[:], xb[:], AF.Tanh)
                for g in range(gsz):
                    m = m0 + g
                    jb, col = divmod(m, 32)
                    st = col == 0
                    sp = col == 31
                    rows = slice(32 * jb, 32 * jb + 32)
                    tp = (0, 32 * jb)
                    nc.tensor.matmul(
                        score[rows, 0:512], z_s[:, 31 - col : 63 - col], tb[:, g, 0:512],
                        start=st, stop=sp, tile_position=tp,
                    )
                    nc.tensor.matmul(
                        score[rows, 512:1024], z_s[:, 31 - col : 63 - col], tb[:, g, 512:1024],
                        start=st, stop=sp, tile_position=tp,
                    )
                    nc.tensor.matmul(
                        score[rows, 1024:1536], z_e[:, 31 - col : 63 - col], tb[:, g, 1024:1536],
                        start=st, stop=sp, tile_position=tp,
                    )
                m0 += gsz

            # ---------- tail-only loads (issued late on purpose) ----------
            wlin = const.tile([128, 3 * H], f32)
            nc.sync.dma_start(wlin[:], din["W_lin"])
            wlinT = const.tile([128, 3, 128], f32)  # [k, a] chunks
            for c in range(3):
                transpose_to(wlinT[:, c, :], wlin[:, c * 128 : (c + 1) * 128], "act")
            blin_c = load_col("b_lin")
            wcoh_c = const.tile([128, 1], f32)
            nc.sync.dma_start(wcoh_c[:], din["W_coh"].rearrange("one p -> p one"))
            bcoh_c = const.tile([1, 1], f32)
            nc.sync.dma_start(bcoh_c[:], din["b_coh"].rearrange("(o t) -> o t", o=1))

            # ---------------- softmax over n (batched across all m) ---------
            # no max subtraction: |score| <= ||ws||_1 ~ 9, exp() safe in fp32.
            # accum_out gives the per-row sum during the same ACTIVATE.
            e_all = work.tile([128, NTOT], f32)
            sum_s = work.tile([128, 1], f32)
            sum_e = work.tile([128, 1], f32)
            nc.scalar.activation(
                e_all[:, 0:NS], score[:, 0:NS], AF.Exp, accum_out=sum_s[:]
            )
            nc.scalar.activation(
                e_all[:, NS:NTOT], score[:, NS:NTOT], AF.Exp, accum_out=sum_e[:]
            )
            rs_s = work.tile([128, 1], f32)
            nc.vector.reciprocal(rs_s[:], sum_s[:])
            rs_e = work.tile([128, 1], f32)
            nc.vector.reciprocal(rs_e[:], sum_e[:])

            # transpose unnormalized weights to [n, m] for ctx matmuls
            esT = work.tile([128, NCH_S, 128], f32)
            for c in range(NCH_S):
                transpose_to(
                    esT[:, c, :], e_all[:, c * 128 : (c + 1) * 128],
                    "act" if c % 2 else "dve",
                )
            eeT = work.tile([128, NCH_E, 128], f32)
            for c in range(NCH_E):
                transpose_to(
                    eeT[:, c, :], e_all[:, NS + c * 128 : NS + (c + 1) * 128],
                    "act" if c % 2 else "dve",
                )

            # ctx[m, h] = (1/sum[m]) * sum_n e[n,m] * attendee[n,h]
            ctx_s_ps = ps_acc.tile([128, 128], f32, tag="ctx_s")
            for c in range(NCH_S):
                nc.tensor.matmul(
                    ctx_s_ps[:], esT[:, c, :], stmts[:, c, :],
                    start=(c == 0), stop=(c == NCH_S - 1),
                )
            ctx_s = work.tile([128, 128], f32)
            nc.vector.tensor_scalar_mul(ctx_s[:], ctx_s_ps[:], rs_s[:])
            ctx_e_ps = ps_acc.tile([128, 128], f32, tag="ctx_e")
            for c in range(NCH_E):
                nc.tensor.matmul(
                    ctx_e_ps[:], eeT[:, c, :], eres[:, c, :],
                    start=(c == 0), stop=(c == NCH_E - 1),
                )
            ctx_e = work.tile([128, 128], f32)
            nc.vector.tensor_scalar_mul(ctx_e[:], ctx_e_ps[:], rs_e[:])

            ctxsT = work.tile([128, 128], f32)
            transpose_to(ctxsT[:], ctx_s[:], "act")
            ctxeT = work.tile([128, 128], f32)
            transpose_to(ctxeT[:], ctx_e[:], "dve")

            # att_vec[a, m] = tanh(sum_k W_linT[k,a] * feats_T[k,m] + b_lin[a])
            av_ps = ps_acc.tile([128, 128], f32, tag="av")
            nc.tensor.matmul(av_ps[:], wlinT[:, 0, :], attT[:], start=True, stop=False)
            nc.tensor.matmul(av_ps[:], wlinT[:, 1, :], ctxsT[:], start=False, stop=False)
            nc.tensor.matmul(av_ps[:], wlinT[:, 2, :], ctxeT[:], start=False, stop=True)
            av = work.tile([128, 128], f32)
            nc.scalar.activation(av[:], av_ps[:], AF.Tanh, bias=blin_c[:])

            # coherence[m] = sum_a W_coh[a] * av[a, m] + b_coh
            coh_ps = ps_acc.tile([1, 128], f32, tag="ctx_s")
            nc.tensor.matmul(coh_ps[:], wcoh_c[:], av[:], start=True, stop=True)
            coh = work.tile([1, 128], f32)
            nc.vector.tensor_scalar_add(coh[:], coh_ps[:], bcoh_c[:])

            nc.sync.dma_start(out_d.rearrange("m one -> one m"), coh[:])

    nc.compile()
    return nc


def _get_nc():
    if "nc" not in _CACHE:
        _CACHE["nc"] = _build_nc()
    return _CACHE["nc"]


def kernel(**inputs):
    from concourse.bass_utils import run_bass_kernel_spmd

    nc = _get_nc()
    full = {k: np.ascontiguousarray(np.asarray(v, dtype=np.float32)) for k, v in inputs.items()}
    in_maps = []
    for i in range(N_CORES):
        m = dict(full)
        m["attender"] = np.ascontiguousarray(
            full["attender"][i * M_LOC : (i + 1) * M_LOC]
        )
        in_maps.append(m)
    res = run_bass_kernel_spmd(nc, in_maps, core_ids=list(range(N_CORES)))
    out = np.concatenate([res.results[i]["out"] for i in range(N_CORES)], axis=0)
    return out.astype(np.float32)


# revision 17
# speedup vs baseline: 1.2683x; 1.0060x over previous
"""CoherenceNet additive-attention kernel for one TRN2 chip (8 NeuronCores).

Problem (per reference):
  score_s[n,m] = ws_s . tanh(A_s[n,:] + B_s[m,:]) + bs_s    (A = stmts@Wc1.T + bc, B = attender@Wc2.T)
  w_ss = softmax over n;  ctx_s = w_ss.T @ stmts             (same for eres)
  att = tanh([attender, ctx_s, ctx_e] @ W_lin.T + b_lin);  out = att @ W_coh.T + b_coh

Sharding: attender (M=1024) axis split across 8 cores (128 attenders per core);
all attendee tensors + weights replicated. No collectives needed - softmax
reduction is over attendees, local to each attender column.

Per-core layout strategy (h/k on partitions for the big middle phase):
  A_sT [h=128, n]   = Wc1 @ stmts.T          (PE, via transposed operands)
  B_sT [h=128, m]   = Wc2 @ att.T + bc       (bias folded here; bs_* cancels in softmax)
  per m: X = A_sT + B_sT[:,m]                (DVE tensor_scalar add, 2x fp32->bf16)
         T = tanh(X)                         (ACT, bf16 - the ~167us/core ACT floor)
         score rows via one-hot ws matmul    (PE bf16, accumulating into PSUM [m,n] layout)
  softmax over n batched across m directly on [m=128, n] PSUM scores (exp without
  max subtraction - scores are bounded by ||ws||_1 so fp32 exp cannot overflow);
  ctx via PE-transposed weights; fp32 MLP head.
"""

import numpy as np

H = 128
NS = 1024
NE = 512
M = 1024
N_CORES = 8
M_LOC = M // N_CORES  # 128 attenders per core
G = 4  # tanh slab batching (m's per ACTIVATE)

_CACHE = {}


def _build_nc():
    import concourse.bacc as bacc
    import concourse.mybir as mybir
    import concourse.tile as tile
    from concourse import masks

    f32 = mybir.dt.float32
    bf16 = mybir.dt.bfloat16
    AF = mybir.ActivationFunctionType
    AX = mybir.AxisListType

    nc = bacc.Bacc(
        "TRN2",
        target_bir_lowering=False,
        debug=False,
        enable_asserts=False,
        num_devices=N_CORES,
    )

    din = {}
    for name, shape in [
        ("attendee_stmts", [NS, H]),
        ("attendee_eres", [NE, H]),
        ("attender", [M_LOC, H]),
        ("Wc_s", [H, 2 * H]),
        ("bc_s", [H]),
        ("ws_s", [H]),
        ("bs_s", [1]),
        ("Wc_e", [H, 2 * H]),
        ("bc_e", [H]),
        ("ws_e", [H]),
        ("bs_e", [1]),
        ("W_lin", [H, 3 * H]),
        ("b_lin", [H]),
        ("W_coh", [1, H]),
        ("b_coh", [1]),
    ]:
        din[name] = nc.dram_tensor(name, shape, f32, kind="ExternalInput").ap()
    out_d = nc.dram_tensor("out", [M_LOC, 1], f32, kind="ExternalOutput").ap()

    NCH_S = NS // 128  # 8 stmt chunks
    NCH_E = NE // 128  # 4 ere chunks
    NTOT = NS + NE  # 1536

    with tile.TileContext(nc) as tc:
        with (
            tc.tile_pool(name="const", bufs=1) as const,
            tc.tile_pool(name="xbuf", bufs=3) as xpool,
            tc.tile_pool(name="tbuf", bufs=3) as tpool,
            tc.tile_pool(name="work", bufs=1) as work,
            tc.tile_pool(name="ps_score", bufs=1, space="PSUM") as ps_score,
            tc.tile_pool(name="ps_tmp", bufs=2, space="PSUM") as ps_tmp,
            tc.tile_pool(name="ps_acc", bufs=1, space="PSUM") as ps_acc,
        ):
            # identity for PE transposes - first, nothing depends on DMA
            ident = const.tile([128, 128], f32)
            masks.make_identity(nc, ident[:])

            def transpose_to(dst_ap, src_ap, copy_eng):
                pt = ps_tmp.tile([128, 128], f32, tag="tmp")
                nc.tensor.transpose(pt[:], src_ap, ident[:])
                if copy_eng == "act":
                    nc.scalar.copy(dst_ap, pt[:])
                else:
                    nc.vector.tensor_copy(dst_ap, pt[:])

            # ---------- critical-path loads (big DMAs on SP queue; each
            # dma_start costs ~650ns of serialized SP issue time, so few+big) --
            stmts = const.tile([128, NCH_S, H], f32)
            stmtsT = const.tile([128, NCH_S, 128], f32)  # [k, n]
            stmts_r = din["attendee_stmts"].rearrange("(c p) h -> p c h", p=128)
            nc.sync.dma_start(stmts[:], stmts_r)
            wc_s = const.tile([128, 2 * H], f32)
            nc.sync.dma_start(wc_s[:], din["Wc_s"])
            att = const.tile([128, H], f32)
            nc.sync.dma_start(att[:], din["attender"])
            eres = const.tile([128, NCH_E, H], f32)
            eres_r = din["attendee_eres"].rearrange("(c p) h -> p c h", p=128)
            nc.sync.dma_start(eres[:], eres_r)
            wc_e = const.tile([128, 2 * H], f32)
            nc.sync.dma_start(wc_e[:], din["Wc_e"])

            def load_col(name):
                t = const.tile([128, 1], f32, tag=f"col_{name}")
                nc.sync.dma_start(
                    t[:], din[name].rearrange("(p one) -> p one", one=1)
                )
                return t

            bc_s_c = load_col("bc_s")
            bc_e_c = load_col("bc_e")
            ws_s_c = load_col("ws_s")
            ws_e_c = load_col("ws_e")

            # ---------- transposes (stmt A path first) ----------
            for c in range(NCH_S):
                transpose_to(stmtsT[:, c, :], stmts[:, c, :], "act" if c % 2 else "dve")
            wc1T_s = const.tile([128, 128], f32)  # [k, h]
            transpose_to(wc1T_s[:], wc_s[:, 0:H], "dve")
            wc2T_s = const.tile([128, 128], f32)
            transpose_to(wc2T_s[:], wc_s[:, H : 2 * H], "dve")
            attT = const.tile([128, 128], f32)  # [k, m]
            transpose_to(attT[:], att[:], "dve")

            # A_sT[h, n] = sum_k Wc1T[k,h] * stmtsT[k,n]
            a_sT = const.tile([128, NS], f32)
            stmtsT_flat = stmtsT[:].rearrange("p c h -> p (c h)")
            for j in range(NS // 512):
                pa = ps_tmp.tile([128, 512], f32, tag="tmp")
                nc.tensor.matmul(
                    pa[:], wc1T_s[:], stmtsT_flat[:, j * 512 : (j + 1) * 512],
                    start=True, stop=True,
                )
                nc.scalar.copy(a_sT[:, j * 512 : (j + 1) * 512], pa[:])
            # B'_sT[h, m] = Wc2T.T @ attT + bc_s
            b_sT = const.tile([128, M_LOC], f32)
            pb = ps_tmp.tile([128, 128], f32, tag="tmp")
            nc.tensor.matmul(pb[:], wc2T_s[:], attT[:], start=True, stop=True)
            nc.vector.tensor_scalar_add(b_sT[:], pb[:], bc_s_c[:])

            # ---------- ere A path ----------
            eresT = const.tile([128, NCH_E, 128], f32)
            for c in range(NCH_E):
                transpose_to(eresT[:, c, :], eres[:, c, :], "act" if c % 2 else "dve")
            wc1T_e = const.tile([128, 128], f32)
            transpose_to(wc1T_e[:], wc_e[:, 0:H], "dve")
            wc2T_e = const.tile([128, 128], f32)
            transpose_to(wc2T_e[:], wc_e[:, H : 2 * H], "dve")

            a_eT = const.tile([128, NE], f32)
            pa = ps_tmp.tile([128, 512], f32, tag="tmp")
            nc.tensor.matmul(
                pa[:], wc1T_e[:], eresT[:].rearrange("p c h -> p (c h)"),
                start=True, stop=True,
            )
            nc.scalar.copy(a_eT[:], pa[:])
            b_eT = const.tile([128, M_LOC], f32)
            pb = ps_tmp.tile([128, 128], f32, tag="tmp")
            nc.tensor.matmul(pb[:], wc2T_e[:], attT[:], start=True, stop=True)
            nc.vector.tensor_scalar_add(b_eT[:], pb[:], bc_e_c[:])

            # one-hot ws strips (bf16): z[:, 0:31]=0, z[:, 31]=ws, z[:, 32:63]=0.
            # The one-hot [128, 32] weight matrix with ws in column c is the
            # contiguous slice z[:, 31-c : 63-c] - no per-column build needed.
            z_s = const.tile([128, 95], bf16)
            z_e = const.tile([128, 63], bf16)
            nc.vector.memset(z_s[:], 0.0)
            nc.vector.memset(z_e[:], 0.0)
            nc.vector.tensor_copy(z_s[:, 31:32], ws_s_c[:])
            nc.vector.tensor_copy(z_e[:, 31:32], ws_e_c[:])

            # ---------------- main loop: tanh slabs + score matmuls ---------
            # scores end up [m=128, n] in PSUM: cols 0:1024 stmt, 1024:1536 ere
            # ramped group sizes: small groups at start (first tanh issues after
            # only 2 adds) and at end (last score MMs trail a small tanh)
            GROUPS = [1, 1, 2] + [G] * ((M_LOC - 8) // G) + [2, 1, 1]
            assert sum(GROUPS) == M_LOC
            score = ps_score.tile([128, NTOT], f32)

            def emit_score_mms(tb, m0, gsz):
                for g in range(gsz):
                    m = m0 + g
                    jb, col = divmod(m, 32)
                    st = col == 0
                    sp = col == 31
                    rows = slice(32 * jb, 32 * jb + 32)
                    tp = (0, 32 * jb)
                    nc.tensor.matmul(
                        score[rows, 0:512], z_s[:, 31 - col : 63 - col],
                        tb[:, g, 0:512], start=st, stop=sp, tile_position=tp,
                    )
                    nc.tensor.matmul(
                        score[rows, 512:1024], z_s[:, 31 - col : 63 - col],
                        tb[:, g, 512:1024], start=st, stop=sp, tile_position=tp,
                    )
                    nc.tensor.matmul(
                        score[rows, 1024:1536], z_e[:, 31 - col : 63 - col],
                        tb[:, g, 1024:1536], start=st, stop=sp, tile_position=tp,
                    )

            m0 = 0
            for gi, gsz in enumerate(GROUPS):
                xb = xpool.tile([128, gsz, NTOT], bf16, tag="xb")
                tb = tpool.tile([128, gsz, NTOT], bf16, tag="tb")
                for g in range(gsz):
                    m = m0 + g
                    nc.vector.tensor_scalar_add(
                        xb[:, g, 0:NS], a_sT[:], b_sT[:, m : m + 1]
                    )
                    if gi > 0:
                        nc.vector.tensor_scalar_add(
                            xb[:, g, NS:NTOT], a_eT[:], b_eT[:, m : m + 1]
                        )
                if gi == 0:
                    # stmt half first: doesn't wait on the ere A/B setup
                    nc.scalar.activation(tb[:, :, 0:NS], xb[:, :, 0:NS], AF.Tanh)
                    for g in range(gsz):
                        m = m0 + g
                        nc.vector.tensor_scalar_add(
                            xb[:, g, NS:NTOT], a_eT[:], b_eT[:, m : m + 1]
                        )
                    nc.scalar.activation(
                        tb[:, :, NS:NTOT], xb[:, :, NS:NTOT], AF.Tanh
                    )
                else:
                    nc.scalar.activation(tb[:], xb[:], AF.Tanh)
                emit_score_mms(tb, m0, gsz)
                m0 += gsz

            # ---------- tail-only loads/casts (issued late on purpose) ------
            wlin = const.tile([128, 3 * H], f32)
            nc.sync.dma_start(wlin[:], din["W_lin"])
            identb = const.tile([128, 128], bf16)
            masks.make_identity(nc, identb[:])

            def transpose_to_bf(dst_ap, src_ap, copy_eng):
                pt = ps_tmp.tile([128, 128], bf16, tag="tmp")
                nc.tensor.transpose(pt[:], src_ap, identb[:])
                if copy_eng == "act":
                    nc.scalar.copy(dst_ap, pt[:])
                else:
                    nc.vector.tensor_copy(dst_ap, pt[:])

            # bf16 copies of tail matmul operands
            stmts_bf = const.tile([128, NCH_S, H], bf16)
            nc.vector.tensor_copy(stmts_bf[:], stmts[:])
            eres_bf = const.tile([128, NCH_E, H], bf16)
            nc.vector.tensor_copy(eres_bf[:], eres[:])
            attT_bf = const.tile([128, 128], bf16)
            nc.vector.tensor_copy(attT_bf[:], attT[:])
            wlin_bf = const.tile([128, 3 * H], bf16)
            nc.vector.tensor_copy(wlin_bf[:], wlin[:])
            wlinT = const.tile([128, 3, 128], bf16)  # [k, a] chunks
            for c in range(3):
                transpose_to_bf(wlinT[:, c, :], wlin_bf[:, c * 128 : (c + 1) * 128], "act")
            wcoh_bf = const.tile([128, 1], bf16)
            blin_c = load_col("b_lin")
            wcoh_c = const.tile([128, 1], f32)
            nc.sync.dma_start(wcoh_c[:], din["W_coh"].rearrange("one p -> p one"))
            bcoh_c = const.tile([1, 1], f32)
            nc.sync.dma_start(bcoh_c[:], din["b_coh"].rearrange("(o t) -> o t", o=1))
            nc.vector.tensor_copy(wcoh_bf[:], wcoh_c[:])

            # ---------------- softmax over n (batched across all m) ---------
            # no max subtraction: |score| <= ||ws||_1 ~ 9, exp() safe in fp32.
            # accum_out gives the per-row sum during the same ACTIVATE.
            # e_all in bf16: the ctx matmuls + transposes then run at 1 cyc/row.
            e_all = work.tile([128, NTOT], bf16)
            sum_s = work.tile([128, 1], f32)
            sum_e = work.tile([128, 1], f32)
            nc.scalar.activation(
                e_all[:, 0:NS], score[:, 0:NS], AF.Exp, accum_out=sum_s[:]
            )
            nc.scalar.activation(
                e_all[:, NS:NTOT], score[:, NS:NTOT], AF.Exp, accum_out=sum_e[:]
            )
            rs_s = work.tile([128, 1], f32)
            nc.vector.reciprocal(rs_s[:], sum_s[:])
            rs_e = work.tile([128, 1], f32)
            nc.vector.reciprocal(rs_e[:], sum_e[:])

            # normalize first (per-partition scale works in [m, n] layout),
            # then transpose to [n, m] for the ctx matmuls
            w_all = work.tile([128, NTOT], bf16)
            nc.vector.tensor_scalar_mul(w_all[:, 0:NS], e_all[:, 0:NS], rs_s[:])
            nc.vector.tensor_scalar_mul(
                w_all[:, NS:NTOT], e_all[:, NS:NTOT], rs_e[:]
            )
            esT = work.tile([128, NCH_S, 128], bf16)
            for c in range(NCH_S):
                transpose_to_bf(
                    esT[:, c, :], w_all[:, c * 128 : (c + 1) * 128],
                    "act" if c % 2 else "dve",
                )
            eeT = work.tile([128, NCH_E, 128], bf16)
            for c in range(NCH_E):
                transpose_to_bf(
                    eeT[:, c, :], w_all[:, NS + c * 128 : NS + (c + 1) * 128],
                    "act" if c % 2 else "dve",
                )

            # ctxT[h, m] = sum_n stmts[n, h] * w[n, m]: stmts chunks are the
            # stationary operand (already in natural [n, h] layout) - no
            # ctx transpose needed, result lands directly featsT-shaped
            ctxs_ps = ps_acc.tile([128, 128], f32, tag="ctx_s")
            for c in range(NCH_S):
                nc.tensor.matmul(
                    ctxs_ps[:], stmts_bf[:, c, :], esT[:, c, :],
                    start=(c == 0), stop=(c == NCH_S - 1),
                )
            ctxsT = work.tile([128, 128], bf16)
            nc.scalar.copy(ctxsT[:], ctxs_ps[:])
            ctxe_ps = ps_acc.tile([128, 128], f32, tag="ctx_e")
            for c in range(NCH_E):
                nc.tensor.matmul(
                    ctxe_ps[:], eres_bf[:, c, :], eeT[:, c, :],
                    start=(c == 0), stop=(c == NCH_E - 1),
                )
            ctxeT = work.tile([128, 128], bf16)
            nc.vector.tensor_copy(ctxeT[:], ctxe_ps[:])

            # att_vec[a, m] = tanh(sum_k W_linT[k,a] * feats_T[k,m] + b_lin[a])
            av_ps = ps_acc.tile([128, 128], f32, tag="av")
            nc.tensor.matmul(av_ps[:], wlinT[:, 0, :], attT_bf[:], start=True, stop=False)
            nc.tensor.matmul(av_ps[:], wlinT[:, 1, :], ctxsT[:], start=False, stop=False)
            nc.tensor.matmul(av_ps[:], wlinT[:, 2, :], ctxeT[:], start=False, stop=True)
            av = work.tile([128, 128], bf16)
            nc.scalar.activation(av[:], av_ps[:], AF.Tanh, bias=blin_c[:])

            # coherence[m] = sum_a W_coh[a] * av[a, m] + b_coh
            coh_ps = ps_acc.tile([1, 128], f32, tag="ctx_s")
            nc.tensor.matmul(coh_ps[:], wcoh_bf[:], av[:], start=True, stop=True)
            coh = work.tile([1, 128], f32)
            nc.vector.tensor_scalar_add(coh[:], coh_ps[:], bcoh_c[:])

            nc.sync.dma_start(out_d.rearrange("m one -> one m"), coh[:])

    nc.compile()
    return nc


def _get_nc():
    if "nc" not in _CACHE:
        _CACHE["nc"] = _build_nc()
    return _CACHE["nc"]


def kernel(**inputs):
    from concourse.bass_utils import run_bass_kernel_spmd

    nc = _get_nc()
    full = {k: np.ascontiguousarray(np.asarray(v, dtype=np.float32)) for k, v in inputs.items()}
    in_maps = []
    for i in range(N_CORES):
        m = dict(full)
        m["attender"] = np.ascontiguousarray(
            full["attender"][i * M_LOC : (i + 1) * M_LOC]
        )
        in_maps.append(m)
    res = run_bass_kernel_spmd(nc, in_maps, core_ids=list(range(N_CORES)))
    out = np.concatenate([res.results[i]["out"] for i in range(N_CORES)], axis=0)
    return out.astype(np.float32)
